# revision 52
# baseline (speedup 1.0000x reference)
"""Block-local self-attention (BLOCK_SIZE=64) Trainium2 Bass kernel, v2.

Full inputs in, full output out. Sharding: batch*heads = 48 planes, 6 planes
per core across 8 cores (pure data parallel, no collectives).

HBM floor is ~12.6 MB/core (Q,K f16 + V_aug bf16 + out f16) ~= 35 us at
358 GB/s; the schedule aims to hide all compute under that DMA curve.

Per-core layout: planes are processed in PAIRS stacked on SBUF partitions
(rows 0:64 = even plane, 64:128 = odd plane). A TILE is 8 query blocks of
one pair = [128, 512] of scores.

Compute per tile (all pair-concurrent via PE array quadrants):
  mm1: per block g, two 64x64x64 matmuls — plane0 in array quadrant
      (rows 0:64, cols 0:64) -> ps1[0:64, g*64:+64], plane1 in quadrant
      (64:128, 64:128) -> ps1[64:128, g*64:+64]. Interleaved issue so the
      two quadrant chains execute concurrently. ps1 = [128,512] = 1 bank,
      DENSE (both planes share columns).
  exp: ONE activation call [128, 512] (contiguous) ps1 -> pt (bf16, SBUF),
      bias = SHIFT (cancels in the softmax ratio).
  mm2: per block g, two concurrent 64-deep matmuls: plane0
      lhsT = pt[0:64, g*64:+64] (keys x queries), rhs = va[0:64, blk, 0:65]
      (V*mask | mask) -> ps2[0:64, g*128 : g*128+65]; plane1 same in the
      opposite quadrant. Column 64 = softmax denominator (ones-column
      trick). ps2 blocks at stride 128 so no 65-col window crosses a PSUM
      bank boundary.
  normalize: reciprocal of denominators [128,8], broadcast-multiply onto
      [128, 8, 64], store f16. (Query-side mask is applied on the HOST
      after unpacking — removes mask DMA + one vector op per tile.)

DMA: three rings so input flow never blocks behind compute-dependent
stores: sync (HWDGE) = Q^T,K^T; scalar (HWDGE) = V_aug (no deps, never
stalls EXP); gpsimd (SWDGE) = output stores (waits only block later
stores). Inputs chunked per (pair, superblock=2 tiles); first unit split
in half so tile 0's deps are minimal. A warmup EXP at t=0 pulls the
~2.7us ACT table load into the DMA ramp.
"""

import numpy as np
import ml_dtypes

BS, H, S, D = 4, 12, 4096, 64
NCORES = 8
PLANES = BS * H          # 48
PPC = PLANES // NCORES   # 6 planes per core
PAIRS = PPC // 2         # 3 plane-pairs per core
NBLK = S // 64           # 64 key/query blocks per plane
NSB = 4                  # superblocks (units) per pair
SHIFT = -20.0            # range shift; cancels in the softmax ratio

_compiled = {}


def _build_nc(ppc=PPC):
    import concourse.bass as bass  # noqa: F401
    import concourse.mybir as mybir
    import concourse.tile as tile
    from concourse import bacc

    f32 = mybir.dt.float32
    bf16 = mybir.dt.bfloat16
    f16 = mybir.dt.float16
    EXP = mybir.ActivationFunctionType.Exp

    pairs = ppc // 2
    UNITS = pairs * NSB  # 12

    nc = bacc.Bacc("TRN2", target_bir_lowering=False, debug=False)

    # q and k packed per (superblock, row): one DMA chunk delivers both
    # operands for 2 tiles as a contiguous 4KB run per partition row.
    qk_d = nc.dram_tensor(
        "qk", [pairs, 128, NSB, 2, 1024], f16, kind="ExternalInput")
    va_d = nc.dram_tensor("vaug", [pairs, 128, NBLK, D + 1], bf16, kind="ExternalInput")
    out_d = nc.dram_tensor("out", [pairs, 128, NBLK, D], f16, kind="ExternalOutput")

    with tile.TileContext(nc) as tc:
        with (
            tc.tile_pool(name="qk", bufs=1) as qk_pool,
            tc.tile_pool(name="vio", bufs=1) as vio_pool,
            tc.tile_pool(name="oio", bufs=1) as oio_pool,
            tc.tile_pool(name="ptp", bufs=1) as pt_pool,
            tc.tile_pool(name="sm", bufs=4) as sm_pool,
            tc.tile_pool(name="ps1", bufs=4, space="PSUM") as ps1_pool,
            tc.tile_pool(name="ps2", bufs=2, space="PSUM") as ps2_pool,
        ):
            bias_u = sm_pool.tile([128, 1], f32, name="bias_u", tag="bias_u", bufs=1)
            nc.vector.memset(bias_u[:], SHIFT)
            # warmup: trigger the ACT exp-table load during the DMA ramp
            wu = sm_pool.tile([128, 1], f32, name="wu", tag="wu", bufs=1)
            nc.scalar.activation(wu[:], bias_u[:], EXP, bias=bias_u[:])

            # Persistent bf16 P^T tiles, block-diagonal per 128-col chunk:
            # off-diagonal quadrants zeroed once and never rewritten, so
            # mm2 can contract all 128 keys (both planes) in ONE matmul
            # per block with a 128-col stationary (FWL-eligible).
            pt_t = []
            for i in range(4):
                t_ = pt_pool.tile([128, 1024], bf16, name=f"pt{i}", tag=f"pt{i}")
                nc.vector.memset(t_[:], 0.0)
                pt_t.append(t_)

            qk_t, va_t, out_t = {}, {}, {}
            for pp in range(pairs):
                qk_t[pp] = qk_pool.tile(
                    [128, NSB, 2, 1024], f16, name=f"qk_t{pp}", tag=f"qk{pp}")
                va_t[pp] = vio_pool.tile(
                    [128, NBLK, D + 1], bf16, name=f"va_t{pp}", tag=f"va{pp}")
                out_t[pp] = oio_pool.tile(
                    [128, NBLK, D], f16, name=f"out_t{pp}", tag=f"out{pp}")

            def issue_inputs_all():
                # sync ring: ALL inputs upfront, consumption order, no
                # waits ever - input flow fully decoupled from compute.
                # qk: one packed 512KB chunk per superblock (q+k for 2
                # tiles, co-arriving under one completion sem); va chunk
                # for the same unit right behind. First and last qk chunks
                # split in half (tile 0 starts early; last tiles' serial
                # compute after the final arrival is short).
                def qk_chunk(pp, sb, splits):
                    for sl in splits:
                        nc.sync.dma_start(
                            qk_t[pp][:, sb, :, sl], qk_d[pp, :, sb, :, sl])

                for u in range(UNITS):
                    pp, sb = divmod(u, NSB)
                    if u == 0 or u == UNITS - 1:
                        qk_chunk(pp, sb, [slice(0, 512), slice(512, 1024)])
                    else:
                        qk_chunk(pp, sb, [slice(0, 1024)])
                    # va in half-pair transfers (fewer DIRECT2Ds on the
                    # ring), placed so each arrives one chunk before its
                    # first consumer
                    if sb in (0, 2):
                        h = sb // 2
                        bsl = slice(h * 32, h * 32 + 32)
                        nc.sync.dma_start(
                            va_t[pp][:, bsl, :], va_d[pp, :, bsl, :])

            ps1_live = {}

            def mm1_exp(t):
                pp, ch = divmod(t, 8)
                sb, half = divmod(ch, 2)
                ps = ps1_pool.tile([128, 512], f32, name="ps1", tag="ps1")
                ps1_live[t] = ps
                cbase = half * 512
                # interleave the two quadrant chains so LDWEIGHTS hides
                # under the opposite quadrant's in-flight matmul
                for g in range(8):
                    c = cbase + g * 64
                    o = g * 64
                    nc.tensor.matmul(
                        ps[0:64, o:o + 64],
                        qk_t[pp][0:64, sb, 1, c:c + 64],
                        qk_t[pp][0:64, sb, 0, c:c + 64],
                        start=True, stop=True)
                    nc.tensor.matmul(
                        ps[64:128, o:o + 64],
                        qk_t[pp][64:128, sb, 1, c:c + 64],
                        qk_t[pp][64:128, sb, 0, c:c + 64],
                        start=True, stop=True)
                pt = pt_t[t % 4]
                ptv = pt[:].rearrange("p (g a b) -> p g a b", g=8, a=2, b=64)
                psv = ps[:].rearrange("p (g b) -> p g b", g=8)
                nc.scalar.activation(
                    ptv[0:64, :, 0, :], psv[0:64, :, :], EXP, bias=bias_u[0:64, :])
                nc.scalar.activation(
                    ptv[64:128, :, 1, :], psv[64:128, :, :], EXP,
                    bias=bias_u[64:128, :])

            def mm2_norm(t):
                pp, ch = divmod(t, 8)
                ps1_live.pop(t)
                pt = pt_t[t % 4]
                ps = ps2_pool.tile([128, 8, 128], f32, name="ps2", tag="ps2")
                b0 = ch * 8
                for g in range(8):
                    o = g * 128
                    nc.tensor.matmul(
                        ps[:, g, 0:65],
                        pt[0:128, o:o + 128],
                        va_t[pp][:, b0 + g, :],
                        start=True, stop=True)
                rc = sm_pool.tile([128, 8], f32, name="rc", tag="rc")
                nc.vector.reciprocal(rc[:], ps[:, :, 64])
                outv = out_t[pp][:, b0:b0 + 8, :]
                rc_b = rc[:].unsqueeze(2).broadcast_to((128, 8, 64))
                nc.vector.tensor_mul(outv, ps[:, :, 0:64], rc_b)

            def out_dma(s):
                # outs ride the SAME sync ring, behind all inputs: FIFO
                # order means inputs drain at full rate first, then outs
                # stream at full rate (norms are far ahead by then).
                # Batched per half-pair (4KB rows) except the final store,
                # which is halved across sync+scalar for drain parallelism.
                pp, sb = divmod(s, NSB)
                b0 = sb * 16
                if s == UNITS - 1:
                    nc.sync.dma_start(
                        out_d[pp, :, b0:b0 + 8, :], out_t[pp][:, b0:b0 + 8, :])
                    nc.scalar.dma_start(
                        out_d[pp, :, b0 + 8:b0 + 16, :],
                        out_t[pp][:, b0 + 8:b0 + 16, :])
                elif s % 2 == 1:
                    # odd superblocks drain through the scalar ring DURING
                    # the run (waits lag norms by a full chunk, so EXPs are
                    # not blocked); even ones flush on sync behind inputs
                    nc.scalar.dma_start(
                        out_d[pp, :, b0:b0 + 16, :], out_t[pp][:, b0:b0 + 16, :])
                else:
                    nc.sync.dma_start(
                        out_d[pp, :, b0:b0 + 16, :], out_t[pp][:, b0:b0 + 16, :])

            issue_inputs_all()
            # chunk-ordered pipeline: per 2-tile chunk, both mm1s (which
            # wait on the chunk's arrival sem) are emitted FIRST, then the
            # always-ready lagged mm2s - so the tensor engine executes the
            # mm2 batch during the next chunk's sem wait instead of the
            # ready work sitting behind the wait in the in-order queue.
            for c in range(UNITS + 1):
                if c < UNITS:
                    mm1_exp(2 * c)
                    mm1_exp(2 * c + 1)
                if c >= 1:
                    mm2_norm(2 * c - 2)
                    mm2_norm(2 * c - 1)
                if c >= 2:
                    out_dma(c - 2)
            out_dma(UNITS - 1)

    nc.compile()
    return nc


def _get_nc(ppc=PPC):
    if ppc not in _compiled:
        _compiled[ppc] = _build_nc(ppc)
    return _compiled[ppc]


def _pack(Q, K, V, mask):
    Qp = np.asarray(Q, np.float32).reshape(PLANES, S, D)
    Kp = np.asarray(K, np.float32).reshape(PLANES, S, D)
    Vp = np.asarray(V, np.float32).reshape(PLANES, S, D)
    maskp = np.asarray(mask, np.float32)[np.repeat(np.arange(BS), H)]  # [48, S]

    # [ncores, pairs, 128, S]: rows 0:64 even plane's d, 64:128 odd plane's d
    qt = np.ascontiguousarray(Qp.transpose(0, 2, 1)).astype(np.float16).reshape(
        NCORES * PAIRS, 128, S)
    kt = np.ascontiguousarray(Kp.transpose(0, 2, 1)).astype(np.float16).reshape(
        NCORES * PAIRS, 128, S)

    # packed [cores, pairs, 128, NSB, 2, 1024]: per (row, superblock) a
    # contiguous qt-chunk then kt-chunk (4KB runs for the DMA)
    qk = np.stack(
        [qt.reshape(-1, 128, NSB, 1024), kt.reshape(-1, 128, NSB, 1024)],
        axis=3)
    qk = np.ascontiguousarray(qk).reshape(NCORES, PAIRS, 128, NSB, 2, 1024)

    # V_aug pair-stacked per block: [pair, r(128), blk, c]; rows 0:64 even
    # plane (seq = 64*blk + r), rows 64:128 odd plane; c 0:64 = V*mask,
    # c 64 = mask (softmax denominator via the mm2 ones-column trick).
    va = np.empty((PLANES, S, D + 1), np.float32)
    va[:, :, :D] = Vp * maskp[:, :, None]
    va[:, :, D] = maskp
    va = va.reshape(PLANES, NBLK, 64, D + 1).transpose(0, 2, 1, 3)  # [pl, r, blk, c]
    va = va.reshape(PLANES // 2, 2 * 64, NBLK, D + 1)  # pair-stack rows
    va = np.ascontiguousarray(va).astype(ml_dtypes.bfloat16).reshape(
        NCORES, PAIRS, 128, NBLK, D + 1)

    return [
        {"qk": qk[c], "vaug": va[c]}
        for c in range(NCORES)
    ]


def _unpack(results, mask):
    # results[c]["out"]: [PAIRS, 128, blk, d]; row r: plane = 2pp + (r>=64),
    # seq = 64*blk + (r % 64)
    full = np.concatenate(
        [results[c]["out"] for c in range(NCORES)], axis=0).astype(np.float32)
    full = full.reshape(PLANES // 2, 2, 64, NBLK, D).transpose(0, 1, 3, 2, 4)
    out = np.ascontiguousarray(full).reshape(BS, H, S, D)
    # query-side mask: zero rows whose query position is masked
    out *= np.asarray(mask, np.float32)[:, None, :, None]
    return out


def run_hw(inputs, trace=False):
    from concourse.bass_utils import run_bass_kernel_spmd

    nc = _get_nc()
    in_maps = _pack(inputs["Q"], inputs["K"], inputs["V"], inputs["mask"])
    res = run_bass_kernel_spmd(nc, in_maps, list(range(NCORES)), trace=trace)
    return _unpack(res.results, inputs["mask"]), res


def kernel(Q, K, V, mask):
    out, _ = run_hw({"Q": Q, "K": K, "V": V, "mask": mask}, trace=False)
    return out
